# revision 20
# baseline (speedup 1.0000x reference)
"""Cross-attention kernel for Trainium2, SPMD over 8 NeuronCores.

Problem: B=2, LQ=1024, LK=10000, E=256, H=8 heads of D=32.
  q = queries @ Wq + bq ; k = bev @ Wk + bk ; v = bev @ Wv + bv
  out = softmax(q k^T) v  @ Wo + bo

Sharding: core c -> (batch b = c // 4, head-pair hp = c % 4).  Each core
computes attention for its 2 heads of its batch plus the partial output
projection through its 64 rows of Wo.  Host sums the 4 partials per batch
and adds bo (plus the bv @ Wo term, see below).

Key structural choices vs a straightforward implementation:
  - Inputs are transposed on the HOST (pure layout staging, zero flops), so
    the device never runs the expensive PE transposes of xq/xk.  All
    energy/projection matmuls are fp32r, which runs at full PE rate for
    N >= 256 moving columns.
  - bk is dropped entirely: a key-side bias adds a per-query constant to
    every energy in the row, which softmax is invariant to (exact).
  - bv is dropped on device: softmax weights sum to 1, so +bv passes
    through attention unchanged; (bv @ Wo) is added into the host-side
    bias along with bo (exact).
  - The softmax denominator comes from an extra all-ones column appended
    to v, so it falls out of the same PE matmuls that compute attn @ v.
  - exp() is split between the Scalar engine (exact table exp) and the
    Vector engine, which computes a Schraudolph-style exp: bf16 bits are
    round(x * 2^7/ln2 + (127*2^7 - 7.35)) as int16.  The -7.35 debias
    makes the approximation mean-preserving so mixing exact and
    approximate k-tiles does not tilt the softmax average.  Per-element
    noise is ~1.8% rms which averages out over the 10k-key weighted sum.
  - Energy matmuls (K=32) run 4-way row-packed; attn@v matmuls (M=33)
    run 2-way column-packed; k/v projections share one stationary matrix
    [Wk | Wv] so they fill all 128 PE columns.
  - Reciprocal of the 2048 denominators is done in a [128, 16] layout
    (tiny PE transposes in, 0-stride-broadcast matmuls out) instead of
    [1, 512] rows, which would run 30x slower on the per-lane DVE.
"""
import sys

sys.path.insert(0, "/opt/trn_rl_repo")

import numpy as np

B, LQ, LK, E, H = 2, 1024, 10000, 256, 8
D = 32            # head dim
HPC = 2           # heads per core
DC = D * HPC      # 64 projected dims per core
LKP = 10240       # LK padded to a multiple of 512
NKT = LKP // 128  # 80 k-tiles
NCH = LKP // 512  # 20 dma chunks

# Schraudolph exp constants (bf16 bits via int16).  The 7.35 debias makes
# E[approx/exp] = 1 over the energy distribution so exact and approximate
# tiles mix without bias.
SCH_A = float(2.0**7 / np.log(2.0))
SCH_B = float(127.0 * 128.0 - 7.35)

# exp engine split pattern per stg group: True = scalar engine (exact),
# False = vector engine (Schraudolph).
SPLIT = (True, True, False, True, False, True, False, True)

_CACHE = {}


def _build():
    import concourse.bacc as bacc
    import concourse.tile as tile
    from concourse import mybir

    FP32 = mybir.dt.float32
    FP32R = mybir.dt.float32r
    BF16 = mybir.dt.bfloat16
    I16 = mybir.dt.int16
    AF = mybir.ActivationFunctionType
    ALU = mybir.AluOpType

    nc = bacc.Bacc("TRN2", target_bir_lowering=False)

    XQT = nc.dram_tensor("xqt", [128, 2, LQ], FP32R, kind="ExternalInput")
    XKT = nc.dram_tensor("xkt", [128, 2, LKP], FP32R, kind="ExternalInput")
    WQ = nc.dram_tensor("wq", [128, 2, DC], FP32R, kind="ExternalInput")
    WKV = nc.dram_tensor("wkv", [128, 2, 128], FP32R, kind="ExternalInput")
    WO = nc.dram_tensor("wo", [32, 2, E], FP32R, kind="ExternalInput")
    BQ = nc.dram_tensor("bq", [DC], FP32, kind="ExternalInput")
    IDT = nc.dram_tensor("ident", [128, 128], FP32, kind="ExternalInput")
    # partial output, transposed: rows = embed dim, cols = query position
    OUT = nc.dram_tensor("out_t", [E, LQ], FP32, kind="ExternalOutput")

    n_grp = [0]

    with tile.TileContext(nc) as tc:
        with (
            tc.tile_pool(name="singles", bufs=1) as sg,
            tc.tile_pool(name="stt", bufs=4) as stp,
            tc.tile_pool(name="wk", bufs=2) as wkp,
            tc.tile_pool(name="avps", bufs=1, space="PSUM") as avp,
        ):
            # ---- constants / weights ----
            ident = sg.tile([128, 128], FP32, tag="ident")
            nc.sync.dma_start(out=ident, in_=IDT[:, :])
            identr = sg.tile([128, 128], FP32R, tag="identr")
            nc.vector.tensor_copy(identr, ident)

            wq_r = sg.tile([128, 2, DC], FP32R, tag="wq")
            nc.sync.dma_start(out=wq_r, in_=WQ[:, :, :])
            wkv_r = sg.tile([128, 2, 128], FP32R, tag="wkv")
            nc.sync.dma_start(out=wkv_r, in_=WKV[:, :, :])
            wo_r = sg.tile([32, 2, E], FP32R, tag="wo")
            nc.sync.dma_start(out=wo_r, in_=WO[:, :, :])
            bq_sb = sg.tile([64, 1], FP32, tag="bq")
            nc.sync.dma_start(out=bq_sb, in_=BQ[:].rearrange("(p o) -> p o", o=1))

            xqT = sg.tile([128, 2, LQ], FP32R, tag="xqT")
            nc.sync.dma_start(out=xqT, in_=XQT[:, :, :])
            xkT = sg.tile([128, 2, LKP], FP32R, tag="xkT")

            # kT/qT rows 0-63 = heads {h0, h1}; rows 64-127 = a copy so the
            # energy matmuls can run 4-way in distinct PE row groups.
            qT = sg.tile([128, LQ], FP32R, tag="qT")
            kT = sg.tile([128, LKP], FP32R, tag="kT")
            v_aug = sg.tile([128, NKT * 66], BF16, tag="vaug")
            # ones columns of v_aug (softmax-denominator trick)
            nc.vector.memset(
                v_aug[:, :].rearrange("p (k t o) -> p k t o", t=2, o=33)[:, :, :, 32:33],
                1.0)

            av = {}
            avs_t = {}

            def warm(st, n):
                # HAM clock-gate feed: only plain-fp32 matmuls register as
                # PE activity, and without sustained counted activity the PE
                # clock stays gated at 1.2 GHz.  These write into stg slots
                # that the next energy matmul overwrites, so they cost no
                # extra PSUM.
                for _ in range(n):
                    nc.tensor.matmul(st[0:32, 0:32], ident[0:32, 0:32],
                                     ident[0:32, 0:32], start=True, stop=True,
                                     skip_group_check=True)

            def emit_unit(kt, h, qc, st, i):
                # one energy matmul into its stg slot (4-way row-packed)
                row = 32 * h + 64 * (kt % 2)
                qs = slice(qc * 512, (qc + 1) * 512)
                nc.tensor.matmul(
                    st[:, i * 512:(i + 1) * 512],
                    kT[row:row + 32, kt * 128:(kt + 1) * 128],
                    qT[row:row + 32, qs],
                    start=True, stop=True, tile_position=(row, 0))

            pending = [None]

            def flush_av():
                # attn@v accumulations for the PREVIOUS group.  Delayed one
                # group so the PE (in-order queue) never reaches them before
                # their exp is done: the current group's energies run while
                # the previous group's exp streams on ACT/DVE.
                if pending[0] is None:
                    return
                units, sT, qc = pending[0]
                pending[0] = None
                for i, (kt, h) in enumerate(units):
                    off = 64 * h
                    # start=True only on the very first write to this bank:
                    # it clears has_written for the WHOLE bank.  Later
                    # matmuls use start=False: overwrite-where-unset /
                    # accumulate-where-set, which is exactly right for the
                    # h0/h1 regions sharing the bank.
                    nc.tensor.matmul(
                        av[qc][off:off + 33, :],
                        v_aug[:, kt * 66 + 33 * h:kt * 66 + 33 * h + 33],
                        sT[:, i * 512:(i + 1) * 512],
                        start=(kt == 0 and h == 0), stop=(kt == NKT - 1),
                        skip_group_check=True)

            def emit_group(units, qc, pool):
                g = n_grp[0]
                n_grp[0] += 1
                st = pool.tile([128, 1024], FP32, tag="stg", name=f"stg{g}")
                if g % 2 == 0:
                    warm(st, 1)
                for i, (kt, h) in enumerate(units):
                    emit_unit(kt, h, qc, st, i)
                w = 512 * len(units)
                sT = stp.tile([128, 1024], BF16, tag="sT", name=f"sT{g}")
                if SPLIT[g % len(SPLIT)]:
                    nc.scalar.activation(sT[:, 0:w], st[:, 0:w], AF.Exp)
                else:
                    nc.vector.tensor_scalar(
                        out=sT[:, 0:w].bitcast(I16), in0=st[:, 0:w],
                        scalar1=SCH_A, scalar2=SCH_B,
                        op0=ALU.mult, op1=ALU.add)
                flush_av()
                pending[0] = (units, sT, qc)

            def evac(qc):
                avs = wkp.tile([33, 1024], FP32, tag="avs", name=f"avs{qc}")
                for h in range(HPC):
                    nc.vector.tensor_copy(avs[:, h * 512:(h + 1) * 512],
                                          av[qc][64 * h:64 * h + 33, :])
                avs_t[qc] = avs

            # =========== single pass: stream chunks, attention for both
            # query halves per chunk (one av bank per half)
            with (
                tc.tile_pool(name="stg0", bufs=2, space="PSUM") as ps0,
                tc.tile_pool(name="kvp", bufs=1, space="PSUM") as kvp,
                tc.tile_pool(name="vpsp", bufs=1, space="PSUM") as vpsp,
            ):
                # q projection first so energies can start with chunk 0
                # (borrows a stg psum tile)
                stq = ps0.tile([128, 1024], FP32, tag="stg", name="stq")
                warm(stq, 80)
                for qc in range(2):
                    qp = stq[0:64, qc * 512:(qc + 1) * 512]
                    for e in range(2):
                        nc.tensor.matmul(qp, wq_r[:, e, :],
                                         xqT[:, e, qc * 512:(qc + 1) * 512],
                                         start=(e == 0), stop=(e == 1))
                    nc.vector.tensor_scalar_add(
                        qT[0:64, qc * 512:(qc + 1) * 512], qp, bq_sb[:, 0:1])
                nc.sync.dma_start(out=qT[64:128, :], in_=qT[0:64, :])

                av[0] = avp.tile([128, 512], FP32, tag="av_0", name="av_q0")
                av[1] = avp.tile([128, 512], FP32, tag="av_1", name="av_q1")

                for c in range(NCH):
                    cs = slice(c * 512, (c + 1) * 512)
                    nc.sync.dma_start(out=xkT[:, :, cs], in_=XKT[:, :, cs])

                    # k/v projection, one stationary [Wk | Wv]
                    kv = kvp.tile([128, 512], FP32, tag="kv", name=f"kv{c}")
                    for e in range(2):
                        nc.tensor.matmul(kv, wkv_r[:, e, :], xkT[:, e, cs],
                                         start=(e == 0), stop=(e == 1))
                    nc.vector.tensor_copy(kT[0:64, cs], kv[0:64, :])
                    nc.sync.dma_start(out=kT[64:128, cs], in_=kT[0:64, cs])

                    # v^T -> v
                    vt = wkp.tile([64, 512], FP32R, tag="vt", name=f"vt{c}")
                    nc.vector.tensor_copy(vt, kv[64:128, :])
                    vps = vpsp.tile([128, 256], FP32R, tag="vps", name=f"vps{c}")
                    for m in range(4):
                        nc.tensor.transpose(
                            vps[:, m * 64:(m + 1) * 64],
                            vt[:, m * 128:(m + 1) * 128],
                            identr[0:64, 0:64])
                    nc.vector.tensor_copy(
                        v_aug[:, c * 264:(c + 1) * 264].rearrange(
                            "p (k t o) -> p k t o", t=2, o=33)[:, :, :, 0:32],
                        vps[:, :].rearrange("p (k t d) -> p k t d", t=2, d=32))

                    # attention for this chunk's 4 k-tiles, both query halves;
                    # kt-pair-major order keeps 4 distinct PE row groups hot
                    for p0 in (0, 2):
                        for qc in range(2):
                            for kt in (c * 4 + p0, c * 4 + p0 + 1):
                                emit_group([(kt, 0), (kt, 1)], qc, ps0)

                flush_av()
            evac(0)
            evac(1)

            # =========== normalize + output projection ----
            attnT = sg.tile([32, 2, LQ], FP32R, tag="attnT")
            out_sb = [sg.tile([128, LQ], FP32, tag=f"out{e}", name=f"out{e}")
                      for e in range(2)]
            rT = sg.tile([128, 16], FP32, tag="rT")

            with tc.tile_pool(name="scp", bufs=2, space="PSUM") as scp:
                # transpose the evac'd accumulators [33, 128]->[128, 33];
                # column 32 of each transposed tile is the denominator slice
                for qc in range(2):
                    avT = scp.tile([128, 264], FP32, tag="avT",
                                   name=f"avT{qc}")
                    for h in range(HPC):
                        for j in range(4):
                            m = h * 4 + j
                            nc.tensor.transpose(
                                avT[:, m * 33:(m + 1) * 33],
                                avs_t[qc][0:33, h * 512 + j * 128:
                                          h * 512 + (j + 1) * 128],
                                ident[0:33, 0:33])
                    nc.vector.reciprocal(
                        rT[:, qc * 8:(qc + 1) * 8],
                        avT[:, :].rearrange("p (m o) -> p m o", o=33)[:, :, 32])

                for qc in range(2):
                    for h in range(HPC):
                        rb = scp.tile([32, 512], FP32, tag="rb",
                                      name=f"rb{qc}{h}")
                        for j in range(4):
                            idx = (qc * 2 + h) * 4 + j
                            nc.tensor.matmul(
                                rb[:, j * 128:(j + 1) * 128],
                                rT[:, idx:idx + 1].broadcast_to((128, 32)),
                                ident, start=True, stop=True)
                        nc.vector.tensor_mul(
                            attnT[:, h, qc * 512:(qc + 1) * 512],
                            avs_t[qc][0:32, h * 512:(h + 1) * 512], rb)

                for ec in range(2):
                    for qc in range(2):
                        po = scp.tile([128, 512], FP32, tag="po",
                                      name=f"po{ec}{qc}")
                        for h in range(HPC):
                            nc.tensor.matmul(
                                po, wo_r[:, h, ec * 128:(ec + 1) * 128],
                                attnT[:, h, qc * 512:(qc + 1) * 512],
                                start=(h == 0), stop=(h == 1))
                        nc.vector.tensor_copy(
                            out_sb[ec][:, qc * 512:(qc + 1) * 512], po)

            for ec in range(2):
                nc.sync.dma_start(out=OUT[ec * 128:(ec + 1) * 128, :],
                                  in_=out_sb[ec])

    nc.compile()
    return nc


def _get_nc():
    if "nc" not in _CACHE:
        _CACHE["nc"] = _build()
    return _CACHE["nc"]


def kernel(bev_emb, queries, Wq, bq, Wk, bk, Wv, bv, Wo, bo):
    from concourse.bass_utils import run_bass_kernel_spmd

    bev_emb = np.asarray(bev_emb, dtype=np.float32)
    queries = np.asarray(queries, dtype=np.float32)
    Wq = np.asarray(Wq, dtype=np.float32)
    bq = np.asarray(bq, dtype=np.float32)
    Wk = np.asarray(Wk, dtype=np.float32)
    bk = np.asarray(bk, dtype=np.float32)
    Wv = np.asarray(Wv, dtype=np.float32)
    bv = np.asarray(bv, dtype=np.float32)
    Wo = np.asarray(Wo, dtype=np.float32)
    bo = np.asarray(bo, dtype=np.float32)

    ident = np.eye(128, dtype=np.float32)

    # host-side layout staging (no flops): transposes + padding
    xqt = []
    xkt = []
    for b in range(B):
        t = np.ascontiguousarray(
            queries[b].T.reshape(2, 128, LQ).transpose(1, 0, 2))
        xqt.append(t)
        kp = np.zeros((128, 2, LKP), dtype=np.float32)
        kp[:, :, :LK] = bev_emb[b].T.reshape(2, 128, LK).transpose(1, 0, 2)
        xkt.append(kp)

    in_maps = []
    for c in range(8):
        b, hp = c // 4, c % 4
        hs = slice(hp * DC, (hp + 1) * DC)
        wkv = np.concatenate([Wk[:, hs], Wv[:, hs]], axis=1)  # [256, 128]
        in_maps.append({
            "xqt": xqt[b],
            "xkt": xkt[b],
            "wq": np.ascontiguousarray(
                Wq[:, hs].reshape(2, 128, DC).transpose(1, 0, 2)),
            "wkv": np.ascontiguousarray(
                wkv.reshape(2, 128, 128).transpose(1, 0, 2)),
            "wo": np.ascontiguousarray(
                Wo[hs, :].reshape(2, 32, E).transpose(1, 0, 2)),
            "bq": np.ascontiguousarray(bq[hs]),
            "ident": ident,
        })

    nc = _get_nc()
    _CACHE["last_in_maps"] = in_maps
    res = run_bass_kernel_spmd(nc, in_maps, list(range(8)))
    _CACHE["last_result"] = res

    out = np.zeros((B, LQ, E), dtype=np.float32)
    for c in range(8):
        out[c // 4] += res.results[c]["out_t"].T
    # bk drops out of softmax exactly; bv rides through attention into the
    # output projection: out += bv @ Wo.  Both folded into the host bias.
    out += bo + bv @ Wo
    return out


# revision 21
# speedup vs baseline: 1.1424x; 1.1424x over previous
"""Cross-attention kernel for Trainium2, SPMD over 8 NeuronCores.

Problem: B=2, LQ=1024, LK=10000, E=256, H=8 heads of D=32.
  q = queries @ Wq + bq ; k = bev @ Wk + bk ; v = bev @ Wv + bv
  out = softmax(q k^T) v  @ Wo + bo

Sharding: core c -> (batch b = c // 4, head-pair hp = c % 4).  Each core
computes attention for its 2 heads of its batch plus the partial output
projection through its 64 rows of Wo.  Host sums the 4 partials per batch
and adds bo (plus the bv @ Wo term, see below).

Key structural choices vs a straightforward implementation:
  - Inputs are transposed on the HOST (pure layout staging, zero flops), so
    the device never runs the expensive PE transposes of xq/xk.  All
    energy/projection matmuls are fp32r, which runs at full PE rate for
    N >= 256 moving columns.
  - bk is dropped entirely: a key-side bias adds a per-query constant to
    every energy in the row, which softmax is invariant to (exact).
  - bv is dropped on device: softmax weights sum to 1, so +bv passes
    through attention unchanged; (bv @ Wo) is added into the host-side
    bias along with bo (exact).
  - The softmax denominator comes from an extra all-ones column appended
    to v, so it falls out of the same PE matmuls that compute attn @ v.
  - exp() is split between the Scalar engine (exact table exp) and the
    Vector engine, which computes a Schraudolph-style exp: bf16 bits are
    round(x * 2^7/ln2 + (127*2^7 - 7.35)) as int16.  The -7.35 debias
    makes the approximation mean-preserving so mixing exact and
    approximate k-tiles does not tilt the softmax average.  Per-element
    noise is ~1.8% rms which averages out over the 10k-key weighted sum.
  - Energy matmuls (K=32) run 4-way row-packed; attn@v matmuls (M=33)
    run 2-way column-packed; k/v projections share one stationary matrix
    [Wk | Wv] so they fill all 128 PE columns.
  - Reciprocal of the 2048 denominators is done in a [128, 16] layout
    (tiny PE transposes in, 0-stride-broadcast matmuls out) instead of
    [1, 512] rows, which would run 30x slower on the per-lane DVE.
"""
import sys

sys.path.insert(0, "/opt/trn_rl_repo")

import numpy as np

B, LQ, LK, E, H = 2, 1024, 10000, 256, 8
D = 32            # head dim
HPC = 2           # heads per core
DC = D * HPC      # 64 projected dims per core
LKP = 10240       # LK padded to a multiple of 512
NKT = LKP // 128  # 80 k-tiles
NCH = LKP // 512  # 20 dma chunks

# Schraudolph exp constants (bf16 bits via int16).  The 7.35 debias makes
# E[approx/exp] = 1 over the energy distribution so exact and approximate
# tiles mix without bias.
SCH_A = float(2.0**7 / np.log(2.0))
SCH_B = float(127.0 * 128.0 - 7.35)

# exp engine split pattern per stg group: True = scalar engine (exact),
# False = vector engine (Schraudolph).
SPLIT = (True, True, False, True, False, True, False, True)

_CACHE = {}


def _build():
    import concourse.bacc as bacc
    import concourse.tile as tile
    from concourse import mybir

    FP32 = mybir.dt.float32
    FP32R = mybir.dt.float32r
    BF16 = mybir.dt.bfloat16
    I16 = mybir.dt.int16
    AF = mybir.ActivationFunctionType
    ALU = mybir.AluOpType

    nc = bacc.Bacc("TRN2", target_bir_lowering=False)

    XQT = nc.dram_tensor("xqt", [128, 2, LQ], FP32R, kind="ExternalInput")
    XKT = nc.dram_tensor("xkt", [128, 2, LKP], FP32R, kind="ExternalInput")
    WQ = nc.dram_tensor("wq", [128, 2, DC], FP32R, kind="ExternalInput")
    WKV = nc.dram_tensor("wkv", [128, 2, 128], FP32R, kind="ExternalInput")
    WO = nc.dram_tensor("wo", [32, 2, E], FP32R, kind="ExternalInput")
    BQ = nc.dram_tensor("bq", [DC], FP32, kind="ExternalInput")
    IDT = nc.dram_tensor("ident", [128, 128], FP32, kind="ExternalInput")
    # partial output, transposed: rows = embed dim, cols = query position
    OUT = nc.dram_tensor("out_t", [E, LQ], FP32, kind="ExternalOutput")

    n_grp = [0]

    with tile.TileContext(nc) as tc:
        with (
            tc.tile_pool(name="singles", bufs=1) as sg,
            tc.tile_pool(name="stt", bufs=4) as stp,
            tc.tile_pool(name="wk", bufs=2) as wkp,
            tc.tile_pool(name="avps", bufs=1, space="PSUM") as avp,
        ):
            # ---- constants / weights ----
            ident = sg.tile([128, 128], FP32, tag="ident")
            nc.sync.dma_start(out=ident, in_=IDT[:, :])
            identr = sg.tile([128, 128], FP32R, tag="identr")
            nc.vector.tensor_copy(identr, ident)

            wq_r = sg.tile([128, 2, DC], FP32R, tag="wq")
            nc.sync.dma_start(out=wq_r, in_=WQ[:, :, :])
            wkv_r = sg.tile([128, 2, 128], FP32R, tag="wkv")
            nc.sync.dma_start(out=wkv_r, in_=WKV[:, :, :])
            wo_r = sg.tile([32, 2, E], FP32R, tag="wo")
            nc.sync.dma_start(out=wo_r, in_=WO[:, :, :])
            bq_sb = sg.tile([64, 1], FP32, tag="bq")
            nc.sync.dma_start(out=bq_sb, in_=BQ[:].rearrange("(p o) -> p o", o=1))

            xqT = sg.tile([128, 2, LQ], FP32R, tag="xqT")
            nc.sync.dma_start(out=xqT, in_=XQT[:, :, :])
            xkT = sg.tile([128, 2, LKP], FP32R, tag="xkT")

            # kT/qT rows 0-63 = heads {h0, h1}; rows 64-127 = a copy so the
            # energy matmuls can run 4-way in distinct PE row groups.
            qT = sg.tile([128, LQ], FP32R, tag="qT")
            kT = sg.tile([128, LKP], FP32R, tag="kT")
            v_aug = sg.tile([128, NKT * 66], BF16, tag="vaug")
            # ones columns of v_aug (softmax-denominator trick)
            nc.vector.memset(
                v_aug[:, :].rearrange("p (k t o) -> p k t o", t=2, o=33)[:, :, :, 32:33],
                1.0)

            av = {}
            avs_t = {}

            def warm(st, n):
                # HAM clock-gate feed: only plain-fp32 matmuls register as
                # PE activity, and without sustained counted activity the PE
                # clock stays gated at 1.2 GHz.  These write into stg slots
                # that the next energy matmul overwrites, so they cost no
                # extra PSUM.
                for _ in range(n):
                    nc.tensor.matmul(st[0:32, 0:32], ident[0:32, 0:32],
                                     ident[0:32, 0:32], start=True, stop=True,
                                     skip_group_check=True)

            def emit_unit(kt, h, qc, st, i):
                # one energy matmul into its stg slot (4-way row-packed)
                row = 32 * h + 64 * (kt % 2)
                qs = slice(qc * 512, (qc + 1) * 512)
                nc.tensor.matmul(
                    st[:, i * 512:(i + 1) * 512],
                    kT[row:row + 32, kt * 128:(kt + 1) * 128],
                    qT[row:row + 32, qs],
                    start=True, stop=True, tile_position=(row, 0))

            pending = []

            def flush_av(depth=2):
                # attn@v accumulations, delayed two groups so the PE
                # (in-order queue) never reaches them before their exp is
                # done: two newer groups' energies run while the exp streams
                # on ACT/DVE.
                if len(pending) < depth:
                    return
                units, sT, qc = pending.pop(0)
                for i, (kt, h) in enumerate(units):
                    off = 64 * h
                    # start=True only on the very first write to this bank:
                    # it clears has_written for the WHOLE bank.  Later
                    # matmuls use start=False: overwrite-where-unset /
                    # accumulate-where-set, which is exactly right for the
                    # h0/h1 regions sharing the bank.
                    nc.tensor.matmul(
                        av[qc][off:off + 33, :],
                        v_aug[:, kt * 66 + 33 * h:kt * 66 + 33 * h + 33],
                        sT[:, i * 512:(i + 1) * 512],
                        start=(kt == 0 and h == 0), stop=(kt == NKT - 1),
                        skip_group_check=True)

            def emit_group(units, qc, pool):
                g = n_grp[0]
                n_grp[0] += 1
                st = pool.tile([128, 1024], FP32, tag="stg", name=f"stg{g}")
                warm(st, 1)
                for i, (kt, h) in enumerate(units):
                    emit_unit(kt, h, qc, st, i)
                w = 512 * len(units)
                sT = stp.tile([128, 1024], BF16, tag="sT", name=f"sT{g}")
                if SPLIT[g % len(SPLIT)]:
                    nc.scalar.activation(sT[:, 0:w], st[:, 0:w], AF.Exp)
                else:
                    nc.vector.tensor_scalar(
                        out=sT[:, 0:w].bitcast(I16), in0=st[:, 0:w],
                        scalar1=SCH_A, scalar2=SCH_B,
                        op0=ALU.mult, op1=ALU.add)
                flush_av(2)
                pending.append((units, sT, qc))

            def evac(qc):
                avs = wkp.tile([33, 1024], FP32, tag="avs", name=f"avs{qc}")
                for h in range(HPC):
                    nc.vector.tensor_copy(avs[:, h * 512:(h + 1) * 512],
                                          av[qc][64 * h:64 * h + 33, :])
                avs_t[qc] = avs

            # =========== single pass: stream chunks, attention for both
            # query halves per chunk (one av bank per half)
            with (
                tc.tile_pool(name="stg0", bufs=2, space="PSUM") as ps0,
                tc.tile_pool(name="kvp", bufs=1, space="PSUM") as kvp,
                tc.tile_pool(name="vpsp", bufs=1, space="PSUM") as vpsp,
            ):
                # q projection first so energies can start with chunk 0
                # (borrows a stg psum tile)
                stq = ps0.tile([128, 1024], FP32, tag="stg", name="stq")
                warm(stq, 80)
                for qc in range(2):
                    qp = stq[0:64, qc * 512:(qc + 1) * 512]
                    for e in range(2):
                        nc.tensor.matmul(qp, wq_r[:, e, :],
                                         xqT[:, e, qc * 512:(qc + 1) * 512],
                                         start=(e == 0), stop=(e == 1))
                    nc.vector.tensor_scalar_add(
                        qT[0:64, qc * 512:(qc + 1) * 512], qp, bq_sb[:, 0:1])
                nc.sync.dma_start(out=qT[64:128, :], in_=qT[0:64, :])

                av[0] = avp.tile([128, 512], FP32, tag="av_0", name="av_q0")
                av[1] = avp.tile([128, 512], FP32, tag="av_1", name="av_q1")

                for c in range(NCH):
                    cs = slice(c * 512, (c + 1) * 512)
                    nc.sync.dma_start(out=xkT[:, :, cs], in_=XKT[:, :, cs])

                    # k/v projection, one stationary [Wk | Wv]
                    kv = kvp.tile([128, 512], FP32, tag="kv", name=f"kv{c}")
                    for e in range(2):
                        nc.tensor.matmul(kv, wkv_r[:, e, :], xkT[:, e, cs],
                                         start=(e == 0), stop=(e == 1))
                    nc.vector.tensor_copy(kT[0:64, cs], kv[0:64, :])
                    nc.sync.dma_start(out=kT[64:128, cs], in_=kT[0:64, cs])

                    # v^T -> v
                    vt = wkp.tile([64, 512], FP32R, tag="vt", name=f"vt{c}")
                    nc.vector.tensor_copy(vt, kv[64:128, :])
                    vps = vpsp.tile([128, 256], FP32R, tag="vps", name=f"vps{c}")
                    for m in range(4):
                        nc.tensor.transpose(
                            vps[:, m * 64:(m + 1) * 64],
                            vt[:, m * 128:(m + 1) * 128],
                            identr[0:64, 0:64])
                    nc.vector.tensor_copy(
                        v_aug[:, c * 264:(c + 1) * 264].rearrange(
                            "p (k t o) -> p k t o", t=2, o=33)[:, :, :, 0:32],
                        vps[:, :].rearrange("p (k t d) -> p k t d", t=2, d=32))

                    # attention for this chunk's 4 k-tiles, both query halves;
                    # kt-pair-major order keeps 4 distinct PE row groups hot
                    for p0 in (0, 2):
                        for qc in range(2):
                            for kt in (c * 4 + p0, c * 4 + p0 + 1):
                                emit_group([(kt, 0), (kt, 1)], qc, ps0)

                while pending:
                    flush_av(1)
            evac(0)
            evac(1)

            # =========== normalize + output projection ----
            attnT = sg.tile([32, 2, LQ], FP32R, tag="attnT")
            out_sb = [sg.tile([128, LQ], FP32, tag=f"out{e}", name=f"out{e}")
                      for e in range(2)]
            rT = sg.tile([128, 16], FP32, tag="rT")

            with tc.tile_pool(name="scp", bufs=2, space="PSUM") as scp:
                # transpose the evac'd accumulators [33, 128]->[128, 33];
                # column 32 of each transposed tile is the denominator slice
                for qc in range(2):
                    avT = scp.tile([128, 264], FP32, tag="avT",
                                   name=f"avT{qc}")
                    for h in range(HPC):
                        for j in range(4):
                            m = h * 4 + j
                            nc.tensor.transpose(
                                avT[:, m * 33:(m + 1) * 33],
                                avs_t[qc][0:33, h * 512 + j * 128:
                                          h * 512 + (j + 1) * 128],
                                ident[0:33, 0:33])
                    nc.vector.reciprocal(
                        rT[:, qc * 8:(qc + 1) * 8],
                        avT[:, :].rearrange("p (m o) -> p m o", o=33)[:, :, 32])

                for qc in range(2):
                    for h in range(HPC):
                        rb = scp.tile([32, 512], FP32, tag="rb",
                                      name=f"rb{qc}{h}")
                        for j in range(4):
                            idx = (qc * 2 + h) * 4 + j
                            nc.tensor.matmul(
                                rb[:, j * 128:(j + 1) * 128],
                                rT[:, idx:idx + 1].broadcast_to((128, 32)),
                                ident, start=True, stop=True)
                        nc.vector.tensor_mul(
                            attnT[:, h, qc * 512:(qc + 1) * 512],
                            avs_t[qc][0:32, h * 512:(h + 1) * 512], rb)

                for ec in range(2):
                    for qc in range(2):
                        po = scp.tile([128, 512], FP32, tag="po",
                                      name=f"po{ec}{qc}")
                        for h in range(HPC):
                            nc.tensor.matmul(
                                po, wo_r[:, h, ec * 128:(ec + 1) * 128],
                                attnT[:, h, qc * 512:(qc + 1) * 512],
                                start=(h == 0), stop=(h == 1))
                        nc.vector.tensor_copy(
                            out_sb[ec][:, qc * 512:(qc + 1) * 512], po)

            for ec in range(2):
                nc.sync.dma_start(out=OUT[ec * 128:(ec + 1) * 128, :],
                                  in_=out_sb[ec])

    nc.compile()
    return nc


def _get_nc():
    if "nc" not in _CACHE:
        _CACHE["nc"] = _build()
    return _CACHE["nc"]


def kernel(bev_emb, queries, Wq, bq, Wk, bk, Wv, bv, Wo, bo):
    from concourse.bass_utils import run_bass_kernel_spmd

    bev_emb = np.asarray(bev_emb, dtype=np.float32)
    queries = np.asarray(queries, dtype=np.float32)
    Wq = np.asarray(Wq, dtype=np.float32)
    bq = np.asarray(bq, dtype=np.float32)
    Wk = np.asarray(Wk, dtype=np.float32)
    bk = np.asarray(bk, dtype=np.float32)
    Wv = np.asarray(Wv, dtype=np.float32)
    bv = np.asarray(bv, dtype=np.float32)
    Wo = np.asarray(Wo, dtype=np.float32)
    bo = np.asarray(bo, dtype=np.float32)

    ident = np.eye(128, dtype=np.float32)

    # host-side layout staging (no flops): transposes + padding
    xqt = []
    xkt = []
    for b in range(B):
        t = np.ascontiguousarray(
            queries[b].T.reshape(2, 128, LQ).transpose(1, 0, 2))
        xqt.append(t)
        kp = np.zeros((128, 2, LKP), dtype=np.float32)
        kp[:, :, :LK] = bev_emb[b].T.reshape(2, 128, LK).transpose(1, 0, 2)
        xkt.append(kp)

    in_maps = []
    for c in range(8):
        b, hp = c // 4, c % 4
        hs = slice(hp * DC, (hp + 1) * DC)
        wkv = np.concatenate([Wk[:, hs], Wv[:, hs]], axis=1)  # [256, 128]
        in_maps.append({
            "xqt": xqt[b],
            "xkt": xkt[b],
            "wq": np.ascontiguousarray(
                Wq[:, hs].reshape(2, 128, DC).transpose(1, 0, 2)),
            "wkv": np.ascontiguousarray(
                wkv.reshape(2, 128, 128).transpose(1, 0, 2)),
            "wo": np.ascontiguousarray(
                Wo[hs, :].reshape(2, 32, E).transpose(1, 0, 2)),
            "bq": np.ascontiguousarray(bq[hs]),
            "ident": ident,
        })

    nc = _get_nc()
    _CACHE["last_in_maps"] = in_maps
    res = run_bass_kernel_spmd(nc, in_maps, list(range(8)))
    _CACHE["last_result"] = res

    out = np.zeros((B, LQ, E), dtype=np.float32)
    for c in range(8):
        out[c // 4] += res.results[c]["out_t"].T
    # bk drops out of softmax exactly; bv rides through attention into the
    # output projection: out += bv @ Wo.  Both folded into the host bias.
    out += bo + bv @ Wo
    return out
